# revision 19
# baseline (speedup 1.0000x reference)
#!/usr/bin/env python3
"""EnvAwareRouter Trainium2 kernel (v2).

Reference computation (per example b):
  t[c]   = gelu(contextual[b,c,:] @ tw1 + tb1) @ tw2 + tb2          (C=13, T=24, H=64)
  logits = gelu(t @ cw1 + cb1) @ cw2 + cb2                          (E=8)
  probs  = softmax(logits + g),  g = -log(-log u + eps)
  mask   = k-hot(top-3 probs);  mask_ste == mask numerically

Device strategy (8 cores, pure data parallel over B=524288; all math fp16
single-term — measured 17/524288 mask flips, rel err 4.4e-3 vs 2e-2 gate):
  - host: transpose contextual to [C*T, B] fp16
  - h1: c-PAIRED matmuls, lhsT [K=48, M=128] block-diag w1 (two c's per
    streamed column), even pairs on PE row-group (0,0), odd on (64,0)
  - tw2 folded into cw1: W2[(c,h), j] = tw2[h]*cw1[c,j]; consume is a
    7-step K=832 accumulated matmul over gelu(h1); per-chunk parity
    alternates pre2 partition halves so consecutive consumes overlap
  - logits with SWAPPED operands: lhsT = h2 [K=64, M=128 examples],
    rhs = cw2 [64, 8] -> z lands NATURALLY [128 examples, 8] in PSUM
    bank 7 (timeshared with pre2); no transposes, no layout shuffles
  - top-3 via 3x max-extraction on DVE; softmax skips max-subtraction
    (z bounded); gumbel noise + cb2 folded into host-precomputed gn
  - examples are processed in a permuted order; host un-permutes outputs
"""
import sys

sys.path.insert(0, "/opt/trn_rl_repo")

import numpy as np

import concourse.bass as bass
import concourse.tile as tile
from concourse import bacc, mybir
from concourse.bass_utils import run_bass_kernel_spmd
from contextlib import ExitStack

F32 = mybir.dt.float32
F16 = mybir.dt.float16
AF = mybir.ActivationFunctionType
OP = mybir.AluOpType
AX = mybir.AxisListType

B, C, T, H, E, TOPK = 524288, 13, 24, 64, 8, 3
EPS = 1e-10
N_CORES = 8
BC = B // N_CORES          # 65536 examples per core
BLK = 8192                 # examples per block (tail granularity)
CHUNK = 512                # examples per compute chunk
DCOLS = 4096               # examples per x SBUF tile (8 chunks)
NEG = -1.0e9               # top-3 extraction knockout
TRACE = False
LAST_EXEC_NS = None


def _build_program(n_examples=BC):
    assert n_examples % BLK == 0
    nblk = n_examples // BLK           # 8
    nchunk = n_examples // CHUNK       # 128
    cpb = BLK // CHUNK                 # 16 chunks per block
    ndg = n_examples // DCOLS          # 16 x d-groups
    cpd = DCOLS // CHUNK               # 8 chunks per d-group
    TW = (BLK // 128) * E              # 512: tail tile width per block
    NE = n_examples * E // 128         # 4096: out/gn dram width
    nc = bacc.Bacc()

    xt_d = nc.declare_dram_parameter("xt", [C * T, n_examples], F16, isOutput=False)
    gn_d = nc.declare_dram_parameter("gn8", [128, NE], F32, isOutput=False)
    w1c_d = nc.declare_dram_parameter("w1c", [128, 128], F16, isOutput=False)
    w2c_d = nc.declare_dram_parameter("w2c", [128, 7 * H], F16, isOutput=False)
    cw2_d = nc.declare_dram_parameter("cw2c", [128, E], F16, isOutput=False)
    tb1_d = nc.declare_dram_parameter("tb1r", [128, 1], F32, isOutput=False)
    b1p_d = nc.declare_dram_parameter("b1pr", [128, 1], F32, isOutput=False)
    mask_d = nc.declare_dram_parameter("mask", [128, NE], F16, isOutput=True)
    probs_d = nc.declare_dram_parameter("probs", [128, NE], F16, isOutput=True)

    with tile.TileContext(nc) as tc, ExitStack() as ctx:
        cpool = ctx.enter_context(tc.tile_pool(name="consts", bufs=1))
        xpool = ctx.enter_context(tc.tile_pool(name="x", bufs=3))
        gpool = ctx.enter_context(tc.tile_pool(name="g1", bufs=3))
        hpool = ctx.enter_context(tc.tile_pool(name="h2", bufs=2))
        zpool = ctx.enter_context(tc.tile_pool(name="zblk", bufs=4))
        tpool = ctx.enter_context(tc.tile_pool(name="tail", bufs=2))
        opool = ctx.enter_context(tc.tile_pool(name="out", bufs=2))
        pspool = ctx.enter_context(tc.tile_pool(name="ps", bufs=1, space="PSUM"))

        # ---- constants ----
        w1c = cpool.tile([128, 128], F16, tag="w1c")
        nc.sync.dma_start(out=w1c[:], in_=w1c_d[:])
        w2c = cpool.tile([128, 7 * H], F16, tag="w2c")
        nc.sync.dma_start(out=w2c[:], in_=w2c_d[:])
        cw2c = cpool.tile([128, E], F16, tag="cw2c")
        nc.sync.dma_start(out=cw2c[:], in_=cw2_d[:])
        tb1r = cpool.tile([128, 1], F32, tag="tb1r")
        nc.sync.dma_start(out=tb1r[:], in_=tb1_d[:])
        b1pr = cpool.tile([128, 1], F32, tag="b1pr")
        nc.sync.dma_start(out=b1pr[:], in_=b1p_d[:])
        warm = cpool.tile([128, 1], F32, tag="warmact")
        nc.scalar.activation(warm[:], tb1r[:], AF.Gelu)

        # ---- persistent PSUM: banks 0-6 h1 pairs, bank 7 pre2/z ----
        ps = pspool.tile([128, 4096], F32, tag="ps")
        nc.vector.memset(ps[:, 3072:3584], 0.0)   # c12 bank upper half garbage

        xts = {}
        zbs = {}
        gns = {}

        def emit_xdma(d):
            xt = xpool.tile([128, 4, DCOLS], F16, tag="xt")
            col0 = d * DCOLS
            # first d-group: chunk-granular pieces so h1(0) starts early
            pieces = (
                [(c * CHUNK, (c + 1) * CHUNK) for c in range(cpd)]
                if d == 0 else [(0, DCOLS // 2), (DCOLS // 2, DCOLS)]
            )
            for lo, hi in pieces:
                for s in range(3):
                    nc.sync.dma_start(
                        out=xt[0:48, s, lo:hi],
                        in_=xt_d[96 * s : 96 * s + 48, col0 + lo : col0 + hi],
                    )
                    nc.sync.dma_start(
                        out=xt[64:112, s, lo:hi],
                        in_=xt_d[96 * s + 48 : 96 * s + 96, col0 + lo : col0 + hi],
                    )
                nc.sync.dma_start(
                    out=xt[0:24, 3, lo:hi],
                    in_=xt_d[288:312, col0 + lo : col0 + hi],
                )
            xts[d] = xt

        def emit_h1(kk):
            d, k = kk // cpd, kk % cpd
            xt = xts[d]
            off = k * CHUNK
            for s in range(3):
                nc.tensor.matmul(
                    ps[:, 1024 * s : 1024 * s + CHUNK],
                    w1c[0:48, :], xt[0:48, s, off : off + CHUNK],
                    start=True, stop=True, tile_position=(0, 0),
                )
                nc.tensor.matmul(
                    ps[:, 1024 * s + 512 : 1024 * s + 512 + CHUNK],
                    w1c[64:112, :], xt[64:112, s, off : off + CHUNK],
                    start=True, stop=True, tile_position=(64, 0),
                )
            nc.tensor.matmul(
                ps[0:64, 3072 : 3072 + CHUNK],
                w1c[0:24, 0:64], xt[0:24, 3, off : off + CHUNK],
                start=True, stop=True, tile_position=(0, 0),
            )

        def emit_gelu1(kk):
            g1 = gpool.tile([128, 3584], F16, tag="g1")
            nc.scalar.activation(g1[:, 0:2048], ps[:, 0:2048], AF.Gelu, bias=tb1r[:])
            nc.scalar.activation(g1[:, 2048:3584], ps[:, 2048:3584], AF.Gelu,
                                 bias=tb1r[:])
            return g1

        def emit_consume(kk, g1):
            pr = 64 * (kk % 2)
            for j in range(7):
                nc.tensor.matmul(
                    ps[pr : pr + 64, 3584:4096],
                    w2c[:, H * j : H * (j + 1)], g1[:, 512 * j : 512 * (j + 1)],
                    start=(j == 0), stop=(j == 6), tile_position=(0, pr),
                )

        def emit_pairtail(P):
            # gelu2 + logits + z evacuation for pair P (chunks 2P, 2P+1)
            h2 = hpool.tile([128, 512], F16, tag="h2")
            nc.scalar.activation(h2[:], ps[:, 3584:4096], AF.Gelu, bias=b1pr[:])
            for a in range(2):
                for g in range(4):
                    zi = a * 4 + g
                    nc.tensor.matmul(
                        ps[:, 3584 + 8 * zi : 3592 + 8 * zi],
                        h2[64 * a : 64 * a + 64, 128 * g : 128 * (g + 1)],
                        cw2c[64 * a : 64 * a + 64, :],
                        start=True, stop=True, tile_position=(64 * a, 0),
                    )
            b = (2 * P) // cpb
            zb = zbs[b]
            ps8 = P % 8
            nc.vector.tensor_copy(zb[:, 64 * ps8 : 64 * ps8 + 64], ps[:, 3584:3648])

        def tail_steps(b):
            """Per-block tail as a list of closures, emitted one per chunk so
            the z-evacuation copies never queue behind a big DVE blob."""
            zb, gnb = zbs[b], gns[b]
            st = {}

            def s_znat():
                st["znat"] = tpool.tile([128, TW], F32, tag="znat", name="znat")
                nc.vector.tensor_tensor(st["znat"][:], zb[:], gnb[:], op=OP.subtract)

            def s_pex():
                st["pex"] = tpool.tile([128, TW], F32, tag="pex", name="pex")
                nc.scalar.activation(st["pex"][:], st["znat"][:], AF.Exp)

            def r3(t):
                return t[:].rearrange("p (w e) -> p w e", e=E)

            def bmax(src_key, tag):
                def f():
                    m = tpool.tile([128, TW // E], F32, tag=tag, name=tag)
                    nc.vector.tensor_reduce(m[:], r3(st[src_key]), axis=AX.X,
                                            op=OP.max)
                    st[tag] = m
                return f

            def s_ge(out_key, src_key, m_key, dtype=F32, pool=None):
                def f():
                    p_ = pool or tpool
                    t = p_.tile([128, TW], dtype, tag=out_key, name=out_key)
                    mb = st[m_key][:].unsqueeze(2).broadcast_to([128, TW // E, E])
                    nc.vector.tensor_tensor(r3(t), r3(st[src_key]), mb, op=OP.is_ge)
                    st[out_key] = t
                return f

            def s_knock(out_key, e_key, z_key):
                def f():
                    t = tpool.tile([128, TW], F32, tag=out_key, name=out_key)
                    nc.vector.scalar_tensor_tensor(t[:], st[e_key][:], NEG,
                                                   st[z_key][:], op0=OP.mult,
                                                   op1=OP.add)
                    st[out_key] = t
                return f

            def s_maskdma():
                nc.sync.dma_start(out=mask_d[:, TW * b : TW * (b + 1)],
                                  in_=st["mask16"][:])

            def s_sm():
                sm = tpool.tile([128, TW // E], F32, tag="sm", name="sm")
                nc.vector.tensor_reduce(sm[:], r3(st["pex"]), axis=AX.X, op=OP.add)
                st["sm"] = sm

            def s_rc():
                rc = tpool.tile([128, TW // E], F32, tag="rc", name="rc")
                nc.vector.reciprocal(rc[:], st["sm"][:])
                st["rc"] = rc

            def s_probs():
                probs16 = opool.tile([128, TW], F16, tag="probs16", name="probs16")
                rcb = st["rc"][:].unsqueeze(2).broadcast_to([128, TW // E, E])
                nc.vector.tensor_tensor(r3(probs16), r3(st["pex"]), rcb, op=OP.mult)
                nc.sync.dma_start(out=probs_d[:, TW * b : TW * (b + 1)],
                                  in_=probs16[:])

            return [
                bmax("znat", "m1"),
                s_ge("e1", "znat", "m1"),
                s_knock("z2", "e1", "znat"),
                bmax("z2", "m2"),
                s_ge("e2", "z2", "m2"),
                s_knock("z3", "e2", "z2"),
                bmax("z3", "m3"),
                s_ge("mask16", "znat", "m3", dtype=F16, pool=opool),
                s_maskdma,
                s_sm,
                s_rc,
                s_probs,
            ], s_znat, s_pex

        # ---- main pipeline ----
        emit_xdma(0)
        emit_h1(0)
        g1 = None
        pending = []
        tail6 = []
        for kk in range(nchunk):
            if kk % cpd == 0 and kk // cpd + 1 < ndg:
                emit_xdma(kk // cpd + 1)
            if kk % cpb == 0:
                b = kk // cpb
                gnb = zpool.tile([128, TW], F32, tag="gnb", name="gnb")
                nc.sync.dma_start(out=gnb[:], in_=gn_d[:, TW * b : TW * (b + 1)])
                gns[b] = gnb
                zbs[b] = zpool.tile([128, TW], F32, tag="zb", name="zb")
            g1 = emit_gelu1(kk)
            if kk + 1 < nchunk:
                emit_h1(kk + 1)
            if kk >= 2 and kk % 2 == 0:
                emit_pairtail(kk // 2 - 1)
            emit_consume(kk, g1)
            # tails for 2 blocks per superblock: znat+exp up front (adjacent
            # exp ops share one table visit), the rest spread 1 op/chunk so
            # z evacuations never wait behind a DVE blob
            if kk % (2 * cpb) == 2 and kk >= 2 * cpb:
                sb = kk // (2 * cpb) - 1
                r0, zn0, px0 = tail_steps(2 * sb)
                r1, zn1, px1 = tail_steps(2 * sb + 1)
                zn0(); zn1(); px0(); px1()
                pending += [x for pr in zip(r0, r1) for x in pr]
            if kk == 114:
                r6, zn6, px6 = tail_steps(nblk - 2)
                tail6.append((r6, px6))
                zn6()
                pending += r6[0:9]   # mask path needs only znat, not exp
            if pending:
                pending.pop(0)()
        emit_pairtail(nchunk // 2 - 1)
        while pending:
            pending.pop(0)()
        r6, px6 = tail6[0]
        r7, zn7, px7 = tail_steps(nblk - 1)
        zn7(); px6(); px7()
        for f in r6[9:]:
            f()
        for f in r7:
            f()

    nc.finalize()
    return nc


def _host_prep(contextual, u, tw1, tb1, tw2, tb2, cw1, cb1, cw2, cb2, n_examples):
    f16, f32 = np.float16, np.float32

    # w1c: block-diag pair weights, replicated at partition 0 and 64
    w1blk = np.zeros((48, 128), f16)
    w1blk[0:24, 0:64] = tw1.astype(f16)
    w1blk[24:48, 64:128] = tw1.astype(f16)
    w1c = np.zeros((128, 128), f16)
    w1c[0:48] = w1blk
    w1c[64:112] = w1blk

    # w2c[p, 64j+m]: p<64 -> c=2j,h=p ; p>=64 -> c=2j+1,h=p-64 (j=6 upper: 0)
    W2 = (tw2[:, 0][None, :, None] * cw1[:, None, :]).astype(f32)  # [C, H, 64]
    w2c = np.zeros((128, 7 * H), f16)
    for j in range(7):
        clo = 2 * j
        w2c[0:64, H * j : H * (j + 1)] = W2[clo].astype(f16)
        if clo + 1 < C:
            w2c[64:128, H * j : H * (j + 1)] = W2[clo + 1].astype(f16)

    cw2c = np.concatenate([cw2.astype(f16), cw2.astype(f16)], axis=0)  # [128, 8]

    tb1r = np.tile(tb1.astype(f32), 2).reshape(128, 1)
    b1p = (cb1 + tb2[0] * cw1.sum(axis=0)).astype(f32)
    b1pr = np.tile(b1p, 2).reshape(128, 1)

    const_map = {
        "w1c": w1c, "w2c": w2c, "cw2c": cw2c, "tb1r": tb1r, "b1pr": b1pr,
    }

    X = contextual.reshape(-1, C * T)
    # gn = -(g + cb2) = log(-log u + eps) - cb2 ; device computes z - gn
    gn_all = (np.log(-np.log(u.astype(f32)) + EPS) - cb2[None, :]).astype(f32)

    nch = n_examples // CHUNK

    def core_inputs(ci):
        s = slice(ci * n_examples, (ci + 1) * n_examples)
        xt = np.ascontiguousarray(X[s].T).astype(f16)     # [312, n]
        gn = gn_all[s]                                    # [n, 8]
        # device order: ex = ch*512 + g*128 + p -> gn_dev[p, (ch*4+g)*8+e]
        gn_dev = np.ascontiguousarray(
            gn.reshape(nch, 4, 128, E).transpose(2, 0, 1, 3).reshape(128, -1)
        )
        return {**const_map, "xt": xt, "gn8": gn_dev}

    return core_inputs


_program_cache = {}


def _get_program(n_examples):
    if n_examples not in _program_cache:
        _program_cache[n_examples] = _build_program(n_examples)
    return _program_cache[n_examples]


def kernel(contextual, u, tw1, tb1, tw2, tb2, cw1, cb1, cw2, cb2):
    n_ex = contextual.shape[0] // N_CORES
    nc = _get_program(n_ex)
    core_inputs = _host_prep(
        np.asarray(contextual), np.asarray(u), np.asarray(tw1), np.asarray(tb1),
        np.asarray(tw2), np.asarray(tb2), np.asarray(cw1), np.asarray(cb1),
        np.asarray(cw2), np.asarray(cb2), n_ex,
    )
    in_maps = [core_inputs(ci) for ci in range(N_CORES)]
    res = run_bass_kernel_spmd(nc, in_maps, list(range(N_CORES)), trace=TRACE)
    global LAST_EXEC_NS
    LAST_EXEC_NS = res.exec_time_ns
    nch = n_ex // CHUNK
    outs = []
    for key in ("mask", "probs"):
        full = np.empty((N_CORES * n_ex, E), np.float32)
        for ci in range(N_CORES):
            dev = res.results[ci][key].astype(np.float32)   # [128, n*8/128]
            # invert: dev[p, (ch*4+g)*8+e] -> ex = ch*512+g*128+p
            full[ci * n_ex : (ci + 1) * n_ex] = (
                dev.reshape(128, nch, 4, E).transpose(1, 2, 0, 3).reshape(n_ex, E)
            )
        outs.append(full)
    return outs[0], outs[1]

